# revision 2
# baseline (speedup 1.0000x reference)
"""LSTM (single layer, final hidden state) on 8 Trainium2 NeuronCores — v3.

Reference computation (per batch row b):
    g     = x_t @ w_ih.T + h @ w_hh.T + (b_ih+b_hh)   # [B, 4H], gates i,f,g,o
    c     = sig(f)*c + sig(i)*tanh(g);  h = sig(o)*tanh(c)

Sharding: data-parallel over batch B=256 -> 8 cores x 32. Weights replicated.

v3 changes vs baseline:
  - float32r matmuls (1 cyc/row at N=256 vs 4 for fp32)
  - x pre-transposed on host, whole xT resident in SBUF (no per-step PE
    transposes / DVE copies / chunked DMA)
  - gate column order (i,f,g_cell,o) with g_cell weight rows scaled x2 on
    host: one sigmoid over [i,f,2g] gives s_g with tanh(g)=2*s_g-1; the o
    sigmoid is issued separately off the critical path
  - fused custom DVE op: u = [s_i|s_f] * select(col<64, 2*[s_g]-1, [c])
    (falls back to tensor_scalar + tensor_tensor if disabled)
  - per-step chain: MM(h x2 rounds) -> sig -> DVE(cup,add) -> tanh ->
    DVE(mul, transpose) -> MM...; bias/x rounds run in PSUM ahead of the
    h rounds, off the critical path.

Per-core layout ("packed"): partition p = 32*j + b, where j in [0,4) indexes
an H-quarter (H index = 64*j + s, s in [0,64)) and b in [0,32) is the local
batch.  Gate tile g_ps [128, 256]: cols 64*q+s with q order (i, f, g, o).
hT (lhsT for the h rounds) via DVE 32x32 block transpose; whh_p host-permuted
to match (hperm).
"""

import os
import sys

import numpy as np

B_TOT, T_FULL, I_DIM, H = 256, 1024, 128, 256
NCORES = 8
B = B_TOT // NCORES  # 32 per core
NJ = 4  # H quarters
S = H // NJ  # 64
# column order within a gate-quarter: (i, f, g_cell, o); row bases in w/b
Q_ROWBASE = (0, 256, 512, 768)
Q_SCALE = (1.0, 1.0, 2.0, 1.0)  # g_cell pre-activation doubled (sig trick)

USE_CUSTOM_DVE = True
WARM_AFTER_X = 0
WARM_AFTER_H = 0


def _ensure_paths():
    for p in ("/opt/trn_rl_repo",):
        if os.path.isdir(p) and p not in sys.path:
            sys.path.append(p)


_CUP_OP = None


def _get_cup_op():
    """Register (once) the fused c-update DVE op:
        out = in0 * select(idx < C0, 2*in1 - 1, in1)
    in0 = [s_i | s_f] (128 cols), in1 = [s_g | c] (128 cols) ->
    out = [s_i * tanh(g) | s_f * c].
    """
    global _CUP_OP
    if _CUP_OP is not None:
        return _CUP_OP
    import concourse.dve_ops as dvo
    from concourse.dve_spec import (
        C0,
        Idx,
        One,
        Spec,
        Src0,
        Src1,
        select,
        lower,
        _has_src1,
    )
    from concourse.dve_uop import DveOpSpec

    for op in dvo.OPS:
        if op.name == "LSTM_CUP":
            _CUP_OP = op
            return op

    def _ref(in0, in1, c0, c1, c2):
        idx = np.arange(in0.shape[-1], dtype=np.float64)
        alt = np.where(idx < c0, 2.0 * in1.astype(np.float64) - 1.0, in1)
        return (in0 * alt).astype(np.float32)

    spec = Spec(
        body=Src0 * select(Idx < C0, Src1 + Src1 - One, Src1),
        reference=_ref,
    )
    opcode = dvo._CUSTOM_DVE_ROW_BASE + len(dvo.OPS)
    assert opcode < 0x20
    shas = {}
    for ver in ("v3", "v4"):
        try:
            tmp = DveOpSpec(
                name="LSTM_CUP",
                opcode=opcode,
                uops=lower(spec, ver=ver),
                rd1_en=_has_src1(spec),
            )
            shas[ver] = tmp.sha(ver)
        except Exception:
            pass
    op = dvo.DveOp("LSTM_CUP", spec, subdim=False, uops_sha=shas, perf_en={"v3": True, "v4": True})
    dvo.OPS.append(op)
    dvo._SUB_OPCODE_FOR_NAME["LSTM_CUP"] = opcode
    _CUP_OP = op
    return op


# DVE 32x32 block-transpose of packed h puts H-input index
# 64*(k//32) + 32*u + (k%32) at partition k of lhsT column-group u.
_K = np.arange(128)
_HPERM = [64 * (_K // 32) + 32 * u + (_K % 32) for u in range(2)]


def _prep_weights(w_ih, w_hh, b_ih, b_hh):
    """Host-side permutation of weights into the packed rhs layouts."""
    wih_p = np.empty((I_DIM, NJ, 4 * S), np.float32)  # [128, 4, 256]
    whh_p = np.empty((128, 2, NJ, 4 * S), np.float32)  # [128, u, j, 256]
    bias_p = np.empty((NJ, 4 * S), np.float32)  # [4, 256] (j on partitions)
    bsum = (b_ih + b_hh).astype(np.float32)
    for q, (rb, sc) in enumerate(zip(Q_ROWBASE, Q_SCALE)):
        for j in range(NJ):
            rows = slice(rb + S * j, rb + S * j + S)
            wih_p[:, j, S * q : S * q + S] = sc * w_ih[rows, :].T
            for u in range(2):
                whh_p[:, u, j, S * q : S * q + S] = sc * w_hh[rows, :][:, _HPERM[u]].T
            bias_p[j, S * q : S * q + S] = sc * bsum[rows]
    # selector: ebias.T @ bias_p broadcasts bias_p[j] to partitions 32j..32j+32
    ebias = np.zeros((NJ, 128), np.float32)
    for j in range(NJ):
        ebias[j, 32 * j : 32 * j + 32] = 1.0
    return wih_p, whh_p, bias_p, ebias


def build_nc(T=T_FULL, debug=False):
    """Build the per-core Bass program (SPMD: same program on all cores)."""
    _ensure_paths()
    import concourse.bacc as bacc
    import concourse.mybir as mybir
    import concourse.tile as tile
    from contextlib import ExitStack

    fp32 = mybir.dt.float32
    f16 = mybir.dt.float16
    AF = mybir.ActivationFunctionType
    ALU = mybir.AluOpType

    nc = bacc.Bacc("TRN2", target_bir_lowering=False, debug=debug)

    xT_d = nc.dram_tensor("xT_p", [I_DIM, T * B], f16, kind="ExternalInput").ap()
    hT0_d = nc.dram_tensor("hT0_p", [128, 2 * 32], f16, kind="ExternalInput").ap()
    c0_d = nc.dram_tensor("c0_p", [128, S], f16, kind="ExternalInput").ap()
    wih_d = nc.dram_tensor("wih_p", [I_DIM, NJ, 4 * S], f16, kind="ExternalInput").ap()
    whh_d = nc.dram_tensor(
        "whh_p", [128, 2, NJ, 4 * S], f16, kind="ExternalInput"
    ).ap()
    bias_d = nc.dram_tensor("bias_p", [NJ, 4 * S], f16, kind="ExternalInput").ap()
    eb_d = nc.dram_tensor("ebias", [NJ, 128], f16, kind="ExternalInput").ap()
    hn_d = nc.dram_tensor("hn", [B, H], fp32, kind="ExternalOutput").ap()

    cup = _get_cup_op() if USE_CUSTOM_DVE else None

    with tile.TileContext(nc) as tc, ExitStack() as ctx:
        consts = ctx.enter_context(tc.tile_pool(name="consts", bufs=1))
        states = ctx.enter_context(tc.tile_pool(name="states", bufs=1))
        hT_pool = ctx.enter_context(tc.tile_pool(name="hT", bufs=3))
        g_psum = ctx.enter_context(tc.tile_pool(name="g_psum", bufs=2, space="PSUM"))
        warm_psum = ctx.enter_context(tc.tile_pool(name="warm_psum", bufs=1, space="PSUM"))

        # ---- constants ----
        wih_sb = consts.tile([I_DIM, NJ, 4 * S], f16, name="wih_sb")
        nc.sync.dma_start(out=wih_sb, in_=wih_d)
        whh_sb = consts.tile([128, 2, NJ, 4 * S], f16, name="whh_sb")
        nc.sync.dma_start(out=whh_sb, in_=whh_d)
        bias_sb = consts.tile([NJ, 4 * S], f16, name="bias_sb")
        nc.sync.dma_start(out=bias_sb, in_=bias_d)
        eb_sb = consts.tile([NJ, 128], f16, name="eb_sb")
        nc.sync.dma_start(out=eb_sb, in_=eb_d)
        ones_sb = consts.tile([1, 32], f16, name="ones_sb")
        nc.vector.memset(ones_sb, 1.0)
        zmv_sb = None
        if WARM_AFTER_X or WARM_AFTER_H:
            zmv_sb = consts.tile([1, 512], f16, name="zmv_sb")
            nc.vector.memset(zmv_sb, 0.0)

        # ---- state ----
        # S-tile: [0:64]=s_i [64:128]=s_f [128:192]=s_g [192:256]=c [256:320]=s_o
        st = states.tile([128, 5 * S], f16, name="st")
        nc.sync.dma_start(out=st[:, 3 * S : 4 * S], in_=c0_d)

        hT_tiles = [
            [
                states.tile([128, 32], f16, name="hT_a_lo"),
                states.tile([128, 32], f16, name="hT_a_hi"),
            ],
            [
                states.tile([128, 32], f16, name="hT_b_lo"),
                states.tile([128, 32], f16, name="hT_b_hi"),
            ],
        ]
        hT = hT_tiles[0]
        nc.sync.dma_start(out=hT[0], in_=hT0_d[:, 0:32])
        nc.sync.dma_start(out=hT[1], in_=hT0_d[:, 32:64])

        # resident pre-transposed x: col index = B*t + b.  Emitted after the
        # small state/weight DMAs and in fine chunks so step 0 is not gated
        # behind the whole 8MB transfer.
        xT_sb = consts.tile([I_DIM, T * B], f16, name="xT_sb")
        NCH = 8
        assert T % NCH == 0
        tch = T // NCH
        for ch in range(NCH):
            nc.sync.dma_start(
                out=xT_sb[:, ch * tch * B : (ch + 1) * tch * B],
                in_=xT_d[:, ch * tch * B : (ch + 1) * tch * B],
            )
        u_sb = states.tile([128, 2 * S], f16, name="u_sb")
        tcc_sb = states.tile([128, S], f16, name="tcc_sb")
        soT_sb = states.tile([128, S], f16, name="soT_sb")
        tccT_a = states.tile([128, 32], f16, name="tccT_a")
        tccT_b = states.tile([128, 32], f16, name="tccT_b")
        h_sb = states.tile([128, S], f16, name="h_sb")
        gt_sb = states.tile([128, S], f16, name="gt_sb")  # fallback path only

        warm_ps = None
        if WARM_AFTER_X or WARM_AFTER_H:
            warm_ps = warm_psum.tile([128, 512], fp32, name="warm_ps")

        def pe_warm(n):
            """Dummy matmuls that keep the PE busy during idle windows."""
            for _ in range(n):
                nc.tensor.matmul(
                    warm_ps[0:32, :], ones_sb, zmv_sb,
                    start=True, stop=True,
                    tile_position=(0, 0), skip_group_check=True,
                )

        for t in range(T):
            g_ps = g_psum.tile([128, 4 * S], fp32, name="g_ps")
            # round-major emission for cross-column-group concurrency
            xT_sl = xT_sb[:, B * t : B * (t + 1)]
            nc.tensor.matmul(
                g_ps, eb_sb, bias_sb,
                start=True, stop=False,
                tile_position=(0, 0), skip_group_check=True,
            )
            for rnd in range(1, 4):
                for j in range(NJ):
                    out = g_ps[32 * j : 32 * j + 32, :]
                    kw = dict(tile_position=(0, 32 * j), skip_group_check=True)
                    if rnd == 1:
                        nc.tensor.matmul(
                            out, xT_sl, wih_sb[:, j, :],
                            start=False, stop=False, **kw,
                        )
                    else:
                        u = rnd - 2
                        nc.tensor.matmul(
                            out,
                            hT[u],
                            whh_sb[:, u, j, :],
                            start=False, stop=(rnd == 3), **kw,
                        )
                if rnd == 1:
                    pe_warm(WARM_AFTER_X)
            pe_warm(WARM_AFTER_H)
            # sig over [i, f, 2g] -> critical; sig over [o] off-path
            nc.scalar.activation(st[:, 0 : 3 * S], g_ps[:, 0 : 3 * S], AF.Sigmoid)
            nc.scalar.activation(st[:, 4 * S : 5 * S], g_ps[:, 3 * S : 4 * S], AF.Sigmoid)
            if cup is not None:
                # u = [s_i * (2*s_g - 1) | s_f * c]
                nc.vector._custom_dve(
                    cup,
                    out=u_sb,
                    in0=st[:, 0 : 2 * S],
                    in1=st[:, 2 * S : 4 * S],
                    s0=float(S),
                )
            else:
                nc.vector.tensor_scalar(
                    gt_sb, st[:, 2 * S : 3 * S], 2.0, -1.0, ALU.mult, ALU.add
                )
                nc.vector.tensor_mul(u_sb[:, 0:S], st[:, 0:S], gt_sb)
                nc.vector.tensor_mul(u_sb[:, S : 2 * S], st[:, S : 2 * S], st[:, 3 * S : 4 * S])
            nc.vector.tensor_add(st[:, 3 * S : 4 * S], u_sb[:, 0:S], u_sb[:, S : 2 * S])
            if t < T - 1:
                # off-path: transpose s_o while tanh(c') runs
                nc.vector.transpose(out=soT_sb, in_=st[:, 4 * S : 5 * S])
            nc.scalar.activation(tcc_sb, st[:, 3 * S : 4 * S], AF.Tanh)
            if t < T - 1:
                hT = hT_tiles[(t + 1) % 2]
                nc.vector.transpose(out=tccT_a, in_=tcc_sb[:, 0:32])
                nc.vector.tensor_mul(hT[0], soT_sb[:, 0:32], tccT_a)
                nc.vector.transpose(out=tccT_b, in_=tcc_sb[:, 32:64])
                nc.vector.tensor_mul(hT[1], soT_sb[:, 32:64], tccT_b)
            else:
                nc.vector.tensor_mul(h_sb, st[:, 4 * S : 5 * S], tcc_sb)

        # ---- write back final h (unpack, bf16 -> fp32) ----
        hfin = states.tile([128, S], fp32, name="hfin")
        nc.vector.tensor_copy(out=hfin, in_=h_sb)
        for j in range(NJ):
            nc.sync.dma_start(
                out=hn_d[:, S * j : S * j + S], in_=hfin[32 * j : 32 * j + 32, :]
            )

    nc.compile()
    return nc


def _shard_inputs(x, h0, c0, w_ih, w_hh, b_ih, b_hh, T=T_FULL):
    bf = np.float16
    wih_p, whh_p, bias_p, ebias = _prep_weights(
        np.asarray(w_ih, np.float32),
        np.asarray(w_hh, np.float32),
        np.asarray(b_ih, np.float32),
        np.asarray(b_hh, np.float32),
    )
    x = np.asarray(x, np.float32)
    h0 = np.asarray(h0, np.float32)
    c0 = np.asarray(c0, np.float32)
    in_maps = []
    for k in range(NCORES):
        bs = slice(B * k, B * (k + 1))
        xc = x[bs, :T, :]  # [32, T, 128]
        # xT_p[i, B*t + b] = x[b, t, i]
        xT_p = np.ascontiguousarray(xc.transpose(2, 1, 0).reshape(I_DIM, T * B))
        h0c = h0[0, bs, :]  # [32, 256]
        c0c = c0[0, bs, :]
        hT0_p = np.empty((128, 64), np.float32)
        for u in range(2):
            hT0_p[:, 32 * u : 32 * u + 32] = h0c[:, _HPERM[u]].T
        c0_p = np.empty((128, S), np.float32)
        for j in range(NJ):
            c0_p[32 * j : 32 * j + 32, :] = c0c[:, S * j : S * j + S]
        in_maps.append(
            {
                "xT_p": xT_p.astype(bf),
                "hT0_p": hT0_p.astype(bf),
                "c0_p": c0_p.astype(bf),
                "wih_p": wih_p.astype(bf),
                "whh_p": whh_p.astype(bf),
                "bias_p": bias_p.astype(bf),
                "ebias": ebias.astype(bf),
            }
        )
    return in_maps


_NC_CACHE = {}


def run_hw(x, h0, c0, w_ih, w_hh, b_ih, b_hh, T=T_FULL, trace=False):
    _ensure_paths()
    from concourse.bass_utils import run_bass_kernel_spmd

    key = (T,)
    if key not in _NC_CACHE:
        _NC_CACHE[key] = build_nc(T=T)
    nc = _NC_CACHE[key]
    in_maps = _shard_inputs(x, h0, c0, w_ih, w_hh, b_ih, b_hh, T=T)
    res = run_bass_kernel_spmd(nc, in_maps, list(range(NCORES)), trace=trace)
    hn = np.stack([res.results[k]["hn"] for k in range(NCORES)], axis=0)
    return hn.reshape(1, B_TOT, H), res


def kernel(x, h0, c0, w_ih, w_hh, b_ih, b_hh):
    out, _ = run_hw(x, h0, c0, w_ih, w_hh, b_ih, b_hh)
    return out.astype(np.float32)


def _np_reference(x, h0, c0, w_ih, w_hh, b_ih, b_hh, T=None):
    """Numpy oracle for development (matches reference.py)."""
    x = np.asarray(x, np.float64)
    if T is not None:
        x = x[:, :T, :]
    h = np.asarray(h0, np.float64)[0]
    c = np.asarray(c0, np.float64)[0]
    gx = np.einsum("bti,gi->tbg", x, np.asarray(w_ih, np.float64)) + (
        np.asarray(b_ih, np.float64) + np.asarray(b_hh, np.float64)
    )
    W = np.asarray(w_hh, np.float64)

    def sg(v):
        return 1.0 / (1.0 + np.exp(-v))

    for t in range(x.shape[1]):
        g = gx[t] + h @ W.T
        i = sg(g[:, 0:256])
        f = sg(g[:, 256:512])
        gg = np.tanh(g[:, 512:768])
        o = sg(g[:, 768:1024])
        c = f * c + i * gg
        h = o * np.tanh(c)
    return h[None].astype(np.float32)


# revision 3
# speedup vs baseline: 1.0012x; 1.0012x over previous
"""LSTM (single layer, final hidden state) on 8 Trainium2 NeuronCores — v3.

Reference computation (per batch row b):
    g     = x_t @ w_ih.T + h @ w_hh.T + (b_ih+b_hh)   # [B, 4H], gates i,f,g,o
    c     = sig(f)*c + sig(i)*tanh(g);  h = sig(o)*tanh(c)

Sharding: data-parallel over batch B=256 -> 8 cores x 32. Weights replicated.

v3 changes vs baseline:
  - float32r matmuls (1 cyc/row at N=256 vs 4 for fp32)
  - x pre-transposed on host, whole xT resident in SBUF (no per-step PE
    transposes / DVE copies / chunked DMA)
  - gate column order (i,f,g_cell,o) with g_cell weight rows scaled x2 on
    host: one sigmoid over [i,f,2g] gives s_g with tanh(g)=2*s_g-1; the o
    sigmoid is issued separately off the critical path
  - fused custom DVE op: u = [s_i|s_f] * select(col<64, 2*[s_g]-1, [c])
    (falls back to tensor_scalar + tensor_tensor if disabled)
  - per-step chain: MM(h x2 rounds) -> sig -> DVE(cup,add) -> tanh ->
    DVE(mul, transpose) -> MM...; bias/x rounds run in PSUM ahead of the
    h rounds, off the critical path.

Per-core layout ("packed"): partition p = 32*j + b, where j in [0,4) indexes
an H-quarter (H index = 64*j + s, s in [0,64)) and b in [0,32) is the local
batch.  Gate tile g_ps [128, 256]: cols 64*q+s with q order (i, f, g, o).
hT (lhsT for the h rounds) via DVE 32x32 block transpose; whh_p host-permuted
to match (hperm).
"""

import os
import sys

import numpy as np

B_TOT, T_FULL, I_DIM, H = 256, 1024, 128, 256
NCORES = 8
B = B_TOT // NCORES  # 32 per core
NJ = 4  # H quarters
S = H // NJ  # 64
# column order within a gate-quarter: (i, f, g_cell, o); row bases in w/b
Q_ROWBASE = (0, 256, 512, 768)
Q_SCALE = (1.0, 1.0, 2.0, 1.0)  # g_cell pre-activation doubled (sig trick)

USE_CUSTOM_DVE = True
WARM_AFTER_X = 0
WARM_AFTER_H = 0


def _ensure_paths():
    for p in ("/opt/trn_rl_repo",):
        if os.path.isdir(p) and p not in sys.path:
            sys.path.append(p)


_CUP_OP = None


def _get_cup_op():
    """Register (once) the fused c-update DVE op:
        out = in0 * select(idx < C0, 2*in1 - 1, in1)
    in0 = [s_i | s_f] (128 cols), in1 = [s_g | c] (128 cols) ->
    out = [s_i * tanh(g) | s_f * c].
    """
    global _CUP_OP
    if _CUP_OP is not None:
        return _CUP_OP
    import concourse.dve_ops as dvo
    from concourse.dve_spec import (
        C0,
        Idx,
        One,
        Spec,
        Src0,
        Src1,
        select,
        lower,
        _has_src1,
    )
    from concourse.dve_uop import DveOpSpec

    for op in dvo.OPS:
        if op.name == "LSTM_CUP":
            _CUP_OP = op
            return op

    def _ref(in0, in1, c0, c1, c2):
        idx = np.arange(in0.shape[-1], dtype=np.float64)
        alt = np.where(idx < c0, 2.0 * in1.astype(np.float64) - 1.0, in1)
        return (in0 * alt).astype(np.float32)

    spec = Spec(
        body=Src0 * select(Idx < C0, Src1 + Src1 - One, Src1),
        reference=_ref,
    )
    opcode = dvo._CUSTOM_DVE_ROW_BASE + len(dvo.OPS)
    assert opcode < 0x20
    shas = {}
    for ver in ("v3", "v4"):
        try:
            tmp = DveOpSpec(
                name="LSTM_CUP",
                opcode=opcode,
                uops=lower(spec, ver=ver),
                rd1_en=_has_src1(spec),
            )
            shas[ver] = tmp.sha(ver)
        except Exception:
            pass
    op = dvo.DveOp("LSTM_CUP", spec, subdim=False, uops_sha=shas, perf_en={"v3": True, "v4": True})
    dvo.OPS.append(op)
    dvo._SUB_OPCODE_FOR_NAME["LSTM_CUP"] = opcode
    _CUP_OP = op
    return op


# DVE 32x32 block-transpose of packed h puts H-input index
# 64*(k//32) + 32*u + (k%32) at partition k of lhsT column-group u.
_K = np.arange(128)
_HPERM = [64 * (_K // 32) + 32 * u + (_K % 32) for u in range(2)]


def _prep_weights(w_ih, w_hh, b_ih, b_hh):
    """Host-side permutation of weights into the packed rhs layouts."""
    wih_p = np.empty((I_DIM, NJ, 4 * S), np.float32)  # [128, 4, 256]
    whh_p = np.empty((128, 2, NJ, 4 * S), np.float32)  # [128, u, j, 256]
    bias_p = np.empty((NJ, 4 * S), np.float32)  # [4, 256] (j on partitions)
    bsum = (b_ih + b_hh).astype(np.float32)
    for q, (rb, sc) in enumerate(zip(Q_ROWBASE, Q_SCALE)):
        for j in range(NJ):
            rows = slice(rb + S * j, rb + S * j + S)
            wih_p[:, j, S * q : S * q + S] = sc * w_ih[rows, :].T
            for u in range(2):
                whh_p[:, u, j, S * q : S * q + S] = sc * w_hh[rows, :][:, _HPERM[u]].T
            bias_p[j, S * q : S * q + S] = sc * bsum[rows]
    # selector: ebias.T @ bias_p broadcasts bias_p[j] to partitions 32j..32j+32
    ebias = np.zeros((NJ, 128), np.float32)
    for j in range(NJ):
        ebias[j, 32 * j : 32 * j + 32] = 1.0
    return wih_p, whh_p, bias_p, ebias


def build_nc(T=T_FULL, debug=False):
    """Build the per-core Bass program (SPMD: same program on all cores)."""
    _ensure_paths()
    import concourse.bacc as bacc
    import concourse.mybir as mybir
    import concourse.tile as tile
    from contextlib import ExitStack

    fp32 = mybir.dt.float32
    f16 = mybir.dt.float16
    AF = mybir.ActivationFunctionType
    ALU = mybir.AluOpType

    nc = bacc.Bacc("TRN2", target_bir_lowering=False, debug=debug)

    xT_d = nc.dram_tensor("xT_p", [I_DIM, T * B], f16, kind="ExternalInput").ap()
    hT0_d = nc.dram_tensor("hT0_p", [128, 2 * 32], f16, kind="ExternalInput").ap()
    c0_d = nc.dram_tensor("c0_p", [128, S], f16, kind="ExternalInput").ap()
    wih_d = nc.dram_tensor("wih_p", [I_DIM, NJ, 4 * S], f16, kind="ExternalInput").ap()
    whh_d = nc.dram_tensor(
        "whh_p", [128, 2, NJ, 4 * S], f16, kind="ExternalInput"
    ).ap()
    bias_d = nc.dram_tensor("bias_p", [NJ, 4 * S], f16, kind="ExternalInput").ap()
    eb_d = nc.dram_tensor("ebias", [NJ, 128], f16, kind="ExternalInput").ap()
    hn_d = nc.dram_tensor("hn", [B, H], fp32, kind="ExternalOutput").ap()

    cup = _get_cup_op() if USE_CUSTOM_DVE else None

    with tile.TileContext(nc) as tc, ExitStack() as ctx:
        consts = ctx.enter_context(tc.tile_pool(name="consts", bufs=1))
        states = ctx.enter_context(tc.tile_pool(name="states", bufs=1))
        hT_pool = ctx.enter_context(tc.tile_pool(name="hT", bufs=3))
        g_psum = ctx.enter_context(tc.tile_pool(name="g_psum", bufs=2, space="PSUM"))
        warm_psum = ctx.enter_context(tc.tile_pool(name="warm_psum", bufs=1, space="PSUM"))

        # ---- constants ----
        wih_sb = consts.tile([I_DIM, NJ, 4 * S], f16, name="wih_sb")
        nc.sync.dma_start(out=wih_sb, in_=wih_d)
        whh_sb = consts.tile([128, 2, NJ, 4 * S], f16, name="whh_sb")
        nc.sync.dma_start(out=whh_sb, in_=whh_d)
        bias_sb = consts.tile([NJ, 4 * S], f16, name="bias_sb")
        nc.sync.dma_start(out=bias_sb, in_=bias_d)
        eb_sb = consts.tile([NJ, 128], f16, name="eb_sb")
        nc.sync.dma_start(out=eb_sb, in_=eb_d)
        ones_sb = consts.tile([1, 32], f16, name="ones_sb")
        nc.vector.memset(ones_sb, 1.0)
        zmv_sb = None
        if WARM_AFTER_X or WARM_AFTER_H:
            zmv_sb = consts.tile([1, 512], f16, name="zmv_sb")
            nc.vector.memset(zmv_sb, 0.0)

        # ---- state ----
        # S-tile: [0:64]=s_i [64:128]=s_f [128:192]=s_g [192:256]=c [256:320]=s_o
        st = states.tile([128, 5 * S], f16, name="st")
        nc.sync.dma_start(out=st[:, 3 * S : 4 * S], in_=c0_d)

        hT_tiles = [
            states.tile([128, 2 * 32], f16, name="hT_a"),
            states.tile([128, 2 * 32], f16, name="hT_b"),
        ]
        hT = hT_tiles[0]
        nc.sync.dma_start(out=hT, in_=hT0_d)

        # resident pre-transposed x: col index = B*t + b.  Emitted after the
        # small state/weight DMAs and in fine chunks so step 0 is not gated
        # behind the whole 8MB transfer.
        xT_sb = consts.tile([I_DIM, T * B], f16, name="xT_sb")
        bounds = [0, min(32, T)]
        while bounds[-1] < T:
            bounds.append(min(bounds[-1] + T // 8, T))
        for lo_t, hi_t in zip(bounds, bounds[1:]):
            nc.sync.dma_start(
                out=xT_sb[:, lo_t * B : hi_t * B],
                in_=xT_d[:, lo_t * B : hi_t * B],
            )
        u_sb = states.tile([128, 2 * S], f16, name="u_sb")
        tcc_sb = states.tile([128, S], f16, name="tcc_sb")
        soT_sb = states.tile([128, S], f16, name="soT_sb")
        tccT_a = states.tile([128, 32], f16, name="tccT_a")
        tccT_b = states.tile([128, 32], f16, name="tccT_b")
        h_sb = states.tile([128, S], f16, name="h_sb")
        gt_sb = states.tile([128, S], f16, name="gt_sb")  # fallback path only

        warm_ps = None
        if WARM_AFTER_X or WARM_AFTER_H:
            warm_ps = warm_psum.tile([128, 512], fp32, name="warm_ps")

        def pe_warm(n):
            """Dummy matmuls that keep the PE busy during idle windows."""
            for _ in range(n):
                nc.tensor.matmul(
                    warm_ps[0:32, :], ones_sb, zmv_sb,
                    start=True, stop=True,
                    tile_position=(0, 0), skip_group_check=True,
                )

        for t in range(T):
            g_ps = g_psum.tile([128, 4 * S], fp32, name="g_ps")
            if t < T - 1:
                # absorb the PE WAR on the next hT buffer off the critical
                # path: this touch carries the "LDWs of step t-1 done" wait so
                # the tail multiplies only need their same-queue RAW wait.
                nc.vector.tensor_copy(
                    out=hT_tiles[(t + 1) % 2][0:1, 0:1], in_=ones_sb[0:1, 0:1]
                )
            # round-major emission for cross-column-group concurrency
            xT_sl = xT_sb[:, B * t : B * (t + 1)]
            nc.tensor.matmul(
                g_ps, eb_sb, bias_sb,
                start=True, stop=False,
                tile_position=(0, 0), skip_group_check=True,
            )
            for rnd in range(1, 4):
                for j in range(NJ):
                    out = g_ps[32 * j : 32 * j + 32, :]
                    kw = dict(tile_position=(0, 32 * j), skip_group_check=True)
                    if rnd == 1:
                        nc.tensor.matmul(
                            out, xT_sl, wih_sb[:, j, :],
                            start=False, stop=False, **kw,
                        )
                    else:
                        u = rnd - 2
                        nc.tensor.matmul(
                            out,
                            hT[:, 32 * u : 32 * u + 32],
                            whh_sb[:, u, j, :],
                            start=False, stop=(rnd == 3), **kw,
                        )
                if rnd == 1:
                    pe_warm(WARM_AFTER_X)
            pe_warm(WARM_AFTER_H)
            # sig over [i, f, 2g] -> critical; sig over [o] off-path
            nc.scalar.activation(st[:, 0 : 3 * S], g_ps[:, 0 : 3 * S], AF.Sigmoid)
            nc.scalar.activation(st[:, 4 * S : 5 * S], g_ps[:, 3 * S : 4 * S], AF.Sigmoid)
            if cup is not None:
                # u = [s_i * (2*s_g - 1) | s_f * c]
                nc.vector._custom_dve(
                    cup,
                    out=u_sb,
                    in0=st[:, 0 : 2 * S],
                    in1=st[:, 2 * S : 4 * S],
                    s0=float(S),
                )
            else:
                nc.vector.tensor_scalar(
                    gt_sb, st[:, 2 * S : 3 * S], 2.0, -1.0, ALU.mult, ALU.add
                )
                nc.vector.tensor_mul(u_sb[:, 0:S], st[:, 0:S], gt_sb)
                nc.vector.tensor_mul(u_sb[:, S : 2 * S], st[:, S : 2 * S], st[:, 3 * S : 4 * S])
            nc.vector.tensor_add(st[:, 3 * S : 4 * S], u_sb[:, 0:S], u_sb[:, S : 2 * S])
            if t < T - 1:
                # off-path: transpose s_o while tanh(c') runs
                nc.vector.transpose(out=soT_sb, in_=st[:, 4 * S : 5 * S])
            nc.scalar.activation(tcc_sb, st[:, 3 * S : 4 * S], AF.Tanh)
            if t < T - 1:
                hT = hT_tiles[(t + 1) % 2]
                nc.vector.transpose(out=tccT_a, in_=tcc_sb[:, 0:32])
                nc.vector.tensor_mul(hT[:, 0:32], soT_sb[:, 0:32], tccT_a)
                nc.vector.transpose(out=tccT_b, in_=tcc_sb[:, 32:64])
                nc.vector.tensor_mul(hT[:, 32:64], soT_sb[:, 32:64], tccT_b)
            else:
                nc.vector.tensor_mul(h_sb, st[:, 4 * S : 5 * S], tcc_sb)

        # ---- write back final h (unpack, bf16 -> fp32) ----
        hfin = states.tile([128, S], fp32, name="hfin")
        nc.vector.tensor_copy(out=hfin, in_=h_sb)
        for j in range(NJ):
            nc.sync.dma_start(
                out=hn_d[:, S * j : S * j + S], in_=hfin[32 * j : 32 * j + 32, :]
            )

    nc.compile()
    return nc


def _shard_inputs(x, h0, c0, w_ih, w_hh, b_ih, b_hh, T=T_FULL):
    bf = np.float16
    wih_p, whh_p, bias_p, ebias = _prep_weights(
        np.asarray(w_ih, np.float32),
        np.asarray(w_hh, np.float32),
        np.asarray(b_ih, np.float32),
        np.asarray(b_hh, np.float32),
    )
    x = np.asarray(x, np.float32)
    h0 = np.asarray(h0, np.float32)
    c0 = np.asarray(c0, np.float32)
    in_maps = []
    for k in range(NCORES):
        bs = slice(B * k, B * (k + 1))
        xc = x[bs, :T, :]  # [32, T, 128]
        # xT_p[i, B*t + b] = x[b, t, i]
        xT_p = np.ascontiguousarray(xc.transpose(2, 1, 0).reshape(I_DIM, T * B))
        h0c = h0[0, bs, :]  # [32, 256]
        c0c = c0[0, bs, :]
        hT0_p = np.empty((128, 64), np.float32)
        for u in range(2):
            hT0_p[:, 32 * u : 32 * u + 32] = h0c[:, _HPERM[u]].T
        c0_p = np.empty((128, S), np.float32)
        for j in range(NJ):
            c0_p[32 * j : 32 * j + 32, :] = c0c[:, S * j : S * j + S]
        in_maps.append(
            {
                "xT_p": xT_p.astype(bf),
                "hT0_p": hT0_p.astype(bf),
                "c0_p": c0_p.astype(bf),
                "wih_p": wih_p.astype(bf),
                "whh_p": whh_p.astype(bf),
                "bias_p": bias_p.astype(bf),
                "ebias": ebias.astype(bf),
            }
        )
    return in_maps


_NC_CACHE = {}


def run_hw(x, h0, c0, w_ih, w_hh, b_ih, b_hh, T=T_FULL, trace=False):
    _ensure_paths()
    from concourse.bass_utils import run_bass_kernel_spmd

    key = (T,)
    if key not in _NC_CACHE:
        _NC_CACHE[key] = build_nc(T=T)
    nc = _NC_CACHE[key]
    in_maps = _shard_inputs(x, h0, c0, w_ih, w_hh, b_ih, b_hh, T=T)
    res = run_bass_kernel_spmd(nc, in_maps, list(range(NCORES)), trace=trace)
    hn = np.stack([res.results[k]["hn"] for k in range(NCORES)], axis=0)
    return hn.reshape(1, B_TOT, H), res


def kernel(x, h0, c0, w_ih, w_hh, b_ih, b_hh):
    out, _ = run_hw(x, h0, c0, w_ih, w_hh, b_ih, b_hh)
    return out.astype(np.float32)


def _np_reference(x, h0, c0, w_ih, w_hh, b_ih, b_hh, T=None):
    """Numpy oracle for development (matches reference.py)."""
    x = np.asarray(x, np.float64)
    if T is not None:
        x = x[:, :T, :]
    h = np.asarray(h0, np.float64)[0]
    c = np.asarray(c0, np.float64)[0]
    gx = np.einsum("bti,gi->tbg", x, np.asarray(w_ih, np.float64)) + (
        np.asarray(b_ih, np.float64) + np.asarray(b_hh, np.float64)
    )
    W = np.asarray(w_hh, np.float64)

    def sg(v):
        return 1.0 / (1.0 + np.exp(-v))

    for t in range(x.shape[1]):
        g = gx[t] + h @ W.T
        i = sg(g[:, 0:256])
        f = sg(g[:, 256:512])
        gg = np.tanh(g[:, 512:768])
        o = sg(g[:, 768:1024])
        c = f * c + i * gg
        h = o * np.tanh(c)
    return h[None].astype(np.float32)


# revision 4
# speedup vs baseline: 1.0870x; 1.0857x over previous
"""LSTM (single layer, final hidden state) on 8 Trainium2 NeuronCores.

Reference computation (per batch row b):
    g     = x_t @ w_ih.T + h @ w_hh.T + (b_ih+b_hh)   # [B, 4H], gates i,f,g,o
    c     = sig(f)*c + sig(i)*tanh(g);  h = sig(o)*tanh(c)

Sharding: data-parallel over batch B=256 -> 8 cores x 32. Weights replicated.
The recurrence is a serial per-step chain; everything is tuned to minimize
that chain's latency:
  - fp16 matmul operands (1 cyc/row; fp32 PSUM accumulate) and fp16
    elementwise state (DVE 2x modes); rel err ~1.2e-3 vs the 2e-2 gate
  - x pre-transposed on host, fully resident in SBUF (no per-step DMA);
    initial-state DMAs issued before the x chunks (prologue ~12us)
  - gate order (i,f,g_cell,o) with g_cell rows scaled x2 on host: one
    sigmoid over [i,f,2g] yields s_g with tanh(g)=2*s_g-1; the o sigmoid
    runs off the critical path
  - custom DVE op fuses the tanh reconstruction with the gate products:
    u = [s_i|s_f] * select(col<64, 2*[s_g]-1, [c])
  - bias enters PSUM as one full-width K=4 matmul (selector x bias rows);
    bias/x rounds run in PSUM ahead of the h rounds, off the chain
  - transposed tail: s_o is transposed during tanh(c); tcc transposes and
    the hT-producing multiplies are split into 32-col halves so the first
    recurrent matmul starts as early as possible
  - a 1-element DVE "touch" of the next hT buffer absorbs the cross-engine
    WAR wait off-path, so the tail multiplies need only their same-queue
    wait (no standalone EVENT_SEMAPHORE stall on the chain)

Per-core layout ("packed"): partition p = 32*j + b, where j in [0,4) indexes
an H-quarter (H index = 64*j + s, s in [0,64)) and b in [0,32) is the local
batch.  Gate tile g_ps [128, 256]: cols 64*q+s with q order (i, f, g, o).
hT (lhsT for the h rounds) via DVE 32x32 block transposes; whh_p
host-permuted to match (hperm).
"""

import os
import sys

import numpy as np

B_TOT, T_FULL, I_DIM, H = 256, 1024, 128, 256
NCORES = 8
B = B_TOT // NCORES  # 32 per core
NJ = 4  # H quarters
S = H // NJ  # 64
# column order within a gate-quarter: (i, f, g_cell, o); row bases in w/b
Q_ROWBASE = (0, 256, 512, 768)
Q_SCALE = (1.0, 1.0, 2.0, 1.0)  # g_cell pre-activation doubled (sig trick)

USE_CUSTOM_DVE = True
WARM_AFTER_X = 0
WARM_AFTER_H = 0


def _ensure_paths():
    for p in ("/opt/trn_rl_repo",):
        if os.path.isdir(p) and p not in sys.path:
            sys.path.append(p)


_CUP_OP = None


def _get_cup_op():
    """Register (once) the fused c-update DVE op:
        out = in0 * select(idx < C0, 2*in1 - 1, in1)
    in0 = [s_i | s_f] (128 cols), in1 = [s_g | c] (128 cols) ->
    out = [s_i * tanh(g) | s_f * c].
    """
    global _CUP_OP
    if _CUP_OP is not None:
        return _CUP_OP
    import concourse.dve_ops as dvo
    from concourse.dve_spec import (
        C0,
        Idx,
        One,
        Spec,
        Src0,
        Src1,
        select,
        lower,
        _has_src1,
    )
    from concourse.dve_uop import DveOpSpec

    for op in dvo.OPS:
        if op.name == "LSTM_CUP":
            _CUP_OP = op
            return op

    def _ref(in0, in1, c0, c1, c2):
        idx = np.arange(in0.shape[-1], dtype=np.float64)
        alt = np.where(idx < c0, 2.0 * in1.astype(np.float64) - 1.0, in1)
        return (in0 * alt).astype(np.float32)

    spec = Spec(
        body=Src0 * select(Idx < C0, Src1 + Src1 - One, Src1),
        reference=_ref,
    )
    opcode = dvo._CUSTOM_DVE_ROW_BASE + len(dvo.OPS)
    assert opcode < 0x20
    shas = {}
    for ver in ("v3", "v4"):
        try:
            tmp = DveOpSpec(
                name="LSTM_CUP",
                opcode=opcode,
                uops=lower(spec, ver=ver),
                rd1_en=_has_src1(spec),
            )
            shas[ver] = tmp.sha(ver)
        except Exception:
            pass
    op = dvo.DveOp("LSTM_CUP", spec, subdim=False, uops_sha=shas, perf_en={"v3": True, "v4": True})
    dvo.OPS.append(op)
    dvo._SUB_OPCODE_FOR_NAME["LSTM_CUP"] = opcode
    _CUP_OP = op
    return op


# DVE 32x32 block-transpose of packed h puts H-input index
# 64*(k//32) + 32*u + (k%32) at partition k of lhsT column-group u.
_K = np.arange(128)
_HPERM = [64 * (_K // 32) + 32 * u + (_K % 32) for u in range(2)]


def _prep_weights(w_ih, w_hh, b_ih, b_hh):
    """Host-side permutation of weights into the packed rhs layouts."""
    wih_p = np.empty((I_DIM, NJ, 4 * S), np.float32)  # [128, 4, 256]
    whh_p = np.empty((128, 2, NJ, 4 * S), np.float32)  # [128, u, j, 256]
    bias_p = np.empty((NJ, 4 * S), np.float32)  # [4, 256] (j on partitions)
    bsum = (b_ih + b_hh).astype(np.float32)
    for q, (rb, sc) in enumerate(zip(Q_ROWBASE, Q_SCALE)):
        for j in range(NJ):
            rows = slice(rb + S * j, rb + S * j + S)
            wih_p[:, j, S * q : S * q + S] = sc * w_ih[rows, :].T
            for u in range(2):
                whh_p[:, u, j, S * q : S * q + S] = sc * w_hh[rows, :][:, _HPERM[u]].T
            bias_p[j, S * q : S * q + S] = sc * bsum[rows]
    # selector: ebias.T @ bias_p broadcasts bias_p[j] to partitions 32j..32j+32
    ebias = np.zeros((NJ, 128), np.float32)
    for j in range(NJ):
        ebias[j, 32 * j : 32 * j + 32] = 1.0
    return wih_p, whh_p, bias_p, ebias


def build_nc(T=T_FULL, debug=False):
    """Build the per-core Bass program (SPMD: same program on all cores)."""
    _ensure_paths()
    import concourse.bacc as bacc
    import concourse.mybir as mybir
    import concourse.tile as tile
    from contextlib import ExitStack

    fp32 = mybir.dt.float32
    f16 = mybir.dt.float16
    AF = mybir.ActivationFunctionType
    ALU = mybir.AluOpType

    nc = bacc.Bacc("TRN2", target_bir_lowering=False, debug=debug)

    xT_d = nc.dram_tensor("xT_p", [I_DIM, T * B], f16, kind="ExternalInput").ap()
    hT0_d = nc.dram_tensor("hT0_p", [128, 2 * 32], f16, kind="ExternalInput").ap()
    c0_d = nc.dram_tensor("c0_p", [128, S], f16, kind="ExternalInput").ap()
    wih_d = nc.dram_tensor("wih_p", [I_DIM, NJ, 4 * S], f16, kind="ExternalInput").ap()
    whh_d = nc.dram_tensor(
        "whh_p", [128, 2, NJ, 4 * S], f16, kind="ExternalInput"
    ).ap()
    bias_d = nc.dram_tensor("bias_p", [NJ, 4 * S], f16, kind="ExternalInput").ap()
    eb_d = nc.dram_tensor("ebias", [NJ, 128], f16, kind="ExternalInput").ap()
    hn_d = nc.dram_tensor("hn", [B, H], fp32, kind="ExternalOutput").ap()

    cup = _get_cup_op() if USE_CUSTOM_DVE else None

    with tile.TileContext(nc) as tc, ExitStack() as ctx:
        consts = ctx.enter_context(tc.tile_pool(name="consts", bufs=1))
        states = ctx.enter_context(tc.tile_pool(name="states", bufs=1))
        hT_pool = ctx.enter_context(tc.tile_pool(name="hT", bufs=3))
        g_psum = ctx.enter_context(tc.tile_pool(name="g_psum", bufs=2, space="PSUM"))
        warm_psum = ctx.enter_context(tc.tile_pool(name="warm_psum", bufs=1, space="PSUM"))

        # ---- constants ----
        wih_sb = consts.tile([I_DIM, NJ, 4 * S], f16, name="wih_sb")
        nc.sync.dma_start(out=wih_sb, in_=wih_d)
        whh_sb = consts.tile([128, 2, NJ, 4 * S], f16, name="whh_sb")
        nc.sync.dma_start(out=whh_sb, in_=whh_d)
        bias_sb = consts.tile([NJ, 4 * S], f16, name="bias_sb")
        nc.sync.dma_start(out=bias_sb, in_=bias_d)
        eb_sb = consts.tile([NJ, 128], f16, name="eb_sb")
        nc.sync.dma_start(out=eb_sb, in_=eb_d)
        ones_sb = consts.tile([1, 32], f16, name="ones_sb")
        nc.vector.memset(ones_sb, 1.0)
        zmv_sb = None
        if WARM_AFTER_X or WARM_AFTER_H:
            zmv_sb = consts.tile([1, 512], f16, name="zmv_sb")
            nc.vector.memset(zmv_sb, 0.0)

        # ---- state ----
        # S-tile: [0:64]=s_i [64:128]=s_f [128:192]=s_g [192:256]=c [256:320]=s_o
        st = states.tile([128, 5 * S], f16, name="st")
        nc.sync.dma_start(out=st[:, 3 * S : 4 * S], in_=c0_d)

        hT_tiles = [
            states.tile([128, 2 * 32], f16, name="hT_a"),
            states.tile([128, 2 * 32], f16, name="hT_b"),
        ]
        hT = hT_tiles[0]
        nc.sync.dma_start(out=hT, in_=hT0_d)

        # resident pre-transposed x: col index = B*t + b.  Emitted after the
        # small state/weight DMAs and in fine chunks so step 0 is not gated
        # behind the whole 8MB transfer.
        xT_sb = consts.tile([I_DIM, T * B], f16, name="xT_sb")
        bounds = [0, min(32, T)]
        while bounds[-1] < T:
            bounds.append(min(bounds[-1] + T // 8, T))
        for lo_t, hi_t in zip(bounds, bounds[1:]):
            nc.sync.dma_start(
                out=xT_sb[:, lo_t * B : hi_t * B],
                in_=xT_d[:, lo_t * B : hi_t * B],
            )
        u_sb = states.tile([128, 2 * S], f16, name="u_sb")
        tcc_sb = states.tile([128, S], f16, name="tcc_sb")
        soT_sb = states.tile([128, S], f16, name="soT_sb")
        tccT_a = states.tile([128, 32], f16, name="tccT_a")
        tccT_b = states.tile([128, 32], f16, name="tccT_b")
        h_sb = states.tile([128, S], f16, name="h_sb")
        gt_sb = states.tile([128, S], f16, name="gt_sb")  # fallback path only

        warm_ps = None
        if WARM_AFTER_X or WARM_AFTER_H:
            warm_ps = warm_psum.tile([128, 512], fp32, name="warm_ps")

        def pe_warm(n):
            """Dummy matmuls that keep the PE busy during idle windows."""
            for _ in range(n):
                nc.tensor.matmul(
                    warm_ps[0:32, :], ones_sb, zmv_sb,
                    start=True, stop=True,
                    tile_position=(0, 0), skip_group_check=True,
                )

        for t in range(T):
            g_ps = g_psum.tile([128, 4 * S], fp32, name="g_ps")
            if t < T - 1:
                # absorb the PE WAR on the next hT buffer off the critical
                # path: this touch carries the "LDWs of step t-1 done" wait so
                # the tail multiplies only need their same-queue RAW wait.
                nc.vector.tensor_copy(
                    out=hT_tiles[(t + 1) % 2][0:1, 0:1], in_=ones_sb[0:1, 0:1]
                )
            # round-major emission for cross-column-group concurrency
            xT_sl = xT_sb[:, B * t : B * (t + 1)]
            nc.tensor.matmul(
                g_ps, eb_sb, bias_sb,
                start=True, stop=False,
                tile_position=(0, 0), skip_group_check=True,
            )
            for rnd in range(1, 4):
                for j in range(NJ):
                    out = g_ps[32 * j : 32 * j + 32, :]
                    kw = dict(tile_position=(0, 32 * j), skip_group_check=True)
                    if rnd == 1:
                        nc.tensor.matmul(
                            out, xT_sl, wih_sb[:, j, :],
                            start=False, stop=False, **kw,
                        )
                    else:
                        u = rnd - 2
                        nc.tensor.matmul(
                            out,
                            hT[:, 32 * u : 32 * u + 32],
                            whh_sb[:, u, j, :],
                            start=False, stop=(rnd == 3), **kw,
                        )
                if rnd == 1:
                    pe_warm(WARM_AFTER_X)
            pe_warm(WARM_AFTER_H)
            # sig over [i, f, 2g] -> critical; sig over [o] off-path
            nc.scalar.activation(st[:, 0 : 3 * S], g_ps[:, 0 : 3 * S], AF.Sigmoid)
            nc.scalar.activation(st[:, 4 * S : 5 * S], g_ps[:, 3 * S : 4 * S], AF.Sigmoid)
            if cup is not None:
                # u = [s_i * (2*s_g - 1) | s_f * c]
                nc.vector._custom_dve(
                    cup,
                    out=u_sb,
                    in0=st[:, 0 : 2 * S],
                    in1=st[:, 2 * S : 4 * S],
                    s0=float(S),
                )
            else:
                nc.vector.tensor_scalar(
                    gt_sb, st[:, 2 * S : 3 * S], 2.0, -1.0, ALU.mult, ALU.add
                )
                nc.vector.tensor_mul(u_sb[:, 0:S], st[:, 0:S], gt_sb)
                nc.vector.tensor_mul(u_sb[:, S : 2 * S], st[:, S : 2 * S], st[:, 3 * S : 4 * S])
            nc.vector.tensor_add(st[:, 3 * S : 4 * S], u_sb[:, 0:S], u_sb[:, S : 2 * S])
            if t < T - 1:
                # off-path: transpose s_o while tanh(c') runs
                nc.vector.transpose(out=soT_sb, in_=st[:, 4 * S : 5 * S])
            nc.scalar.activation(tcc_sb, st[:, 3 * S : 4 * S], AF.Tanh)
            if t < T - 1:
                hT = hT_tiles[(t + 1) % 2]
                nc.vector.transpose(out=tccT_a, in_=tcc_sb[:, 0:32])
                nc.vector.tensor_mul(hT[:, 0:32], soT_sb[:, 0:32], tccT_a)
                nc.vector.transpose(out=tccT_b, in_=tcc_sb[:, 32:64])
                nc.vector.tensor_mul(hT[:, 32:64], soT_sb[:, 32:64], tccT_b)
            else:
                nc.vector.tensor_mul(h_sb, st[:, 4 * S : 5 * S], tcc_sb)

        # ---- write back final h (unpack, bf16 -> fp32) ----
        hfin = states.tile([128, S], fp32, name="hfin")
        nc.vector.tensor_copy(out=hfin, in_=h_sb)
        for j in range(NJ):
            nc.sync.dma_start(
                out=hn_d[:, S * j : S * j + S], in_=hfin[32 * j : 32 * j + 32, :]
            )

    nc.compile()
    return nc


def _shard_inputs(x, h0, c0, w_ih, w_hh, b_ih, b_hh, T=T_FULL):
    bf = np.float16
    wih_p, whh_p, bias_p, ebias = _prep_weights(
        np.asarray(w_ih, np.float32),
        np.asarray(w_hh, np.float32),
        np.asarray(b_ih, np.float32),
        np.asarray(b_hh, np.float32),
    )
    x = np.asarray(x, np.float32)
    h0 = np.asarray(h0, np.float32)
    c0 = np.asarray(c0, np.float32)
    in_maps = []
    for k in range(NCORES):
        bs = slice(B * k, B * (k + 1))
        xc = x[bs, :T, :]  # [32, T, 128]
        # xT_p[i, B*t + b] = x[b, t, i]
        xT_p = np.ascontiguousarray(xc.transpose(2, 1, 0).reshape(I_DIM, T * B))
        h0c = h0[0, bs, :]  # [32, 256]
        c0c = c0[0, bs, :]
        hT0_p = np.empty((128, 64), np.float32)
        for u in range(2):
            hT0_p[:, 32 * u : 32 * u + 32] = h0c[:, _HPERM[u]].T
        c0_p = np.empty((128, S), np.float32)
        for j in range(NJ):
            c0_p[32 * j : 32 * j + 32, :] = c0c[:, S * j : S * j + S]
        in_maps.append(
            {
                "xT_p": xT_p.astype(bf),
                "hT0_p": hT0_p.astype(bf),
                "c0_p": c0_p.astype(bf),
                "wih_p": wih_p.astype(bf),
                "whh_p": whh_p.astype(bf),
                "bias_p": bias_p.astype(bf),
                "ebias": ebias.astype(bf),
            }
        )
    return in_maps


_NC_CACHE = {}


def run_hw(x, h0, c0, w_ih, w_hh, b_ih, b_hh, T=T_FULL, trace=False):
    _ensure_paths()
    from concourse.bass_utils import run_bass_kernel_spmd

    key = (T,)
    if key not in _NC_CACHE:
        _NC_CACHE[key] = build_nc(T=T)
    nc = _NC_CACHE[key]
    in_maps = _shard_inputs(x, h0, c0, w_ih, w_hh, b_ih, b_hh, T=T)
    res = run_bass_kernel_spmd(nc, in_maps, list(range(NCORES)), trace=trace)
    hn = np.stack([res.results[k]["hn"] for k in range(NCORES)], axis=0)
    return hn.reshape(1, B_TOT, H), res


def kernel(x, h0, c0, w_ih, w_hh, b_ih, b_hh):
    out, _ = run_hw(x, h0, c0, w_ih, w_hh, b_ih, b_hh)
    return out.astype(np.float32)


def _np_reference(x, h0, c0, w_ih, w_hh, b_ih, b_hh, T=None):
    """Numpy oracle for development (matches reference.py)."""
    x = np.asarray(x, np.float64)
    if T is not None:
        x = x[:, :T, :]
    h = np.asarray(h0, np.float64)[0]
    c = np.asarray(c0, np.float64)[0]
    gx = np.einsum("bti,gi->tbg", x, np.asarray(w_ih, np.float64)) + (
        np.asarray(b_ih, np.float64) + np.asarray(b_hh, np.float64)
    )
    W = np.asarray(w_hh, np.float64)

    def sg(v):
        return 1.0 / (1.0 + np.exp(-v))

    for t in range(x.shape[1]):
        g = gx[t] + h @ W.T
        i = sg(g[:, 0:256])
        f = sg(g[:, 256:512])
        gg = np.tanh(g[:, 512:768])
        o = sg(g[:, 768:1024])
        c = f * c + i * gg
        h = o * np.tanh(c)
    return h[None].astype(np.float32)
